# revision 13
# baseline (speedup 1.0000x reference)
"""Trainium2 Bass kernel for nn_Discriminator (5-layer GRU stack + MLPs + head).

Data-parallel over 8 NeuronCores (batch 2048 -> 256/core).

Math/layout notes (device program):
  - All activations kept "transposed": [feature_rows, batch_cols=256] in SBUF.
  - GRU state stored as S = 2*h  (rows 0:40), with row 40 = constant 1.0
    (ones row feeds biases through matmuls).
  - sigmoid(x) computed as (tanh(x/2)+1)/2: the rz pre-activations are built
    *pre-halved* by scaling weights, so only Tanh is ever evaluated; the
    (t+1)/2 affine is folded into the gate algebra below.
  - Engine access patterns require 32-aligned partition bases, so gate rows
    are placed at bases {0, 64}:  psum bank per layer [128, 512]:
       cols 0:256  : r-preact rows 0:40, z-preact rows 64:104
       cols 256:512: xn+bin   rows 0:40, p=(hn+bhn)/2 rows 64:104
  - Per GRU layer & step, with tau = tanh(halved rz preact):
        v   = (tau_r + 1) * p                 = r*(hn+bhn)     [STT]
        t2  = v + (xn + bin)
        n   = tanh(t2)
        e   = 0.5*S - n                       = h - n          [STT]
        u   = (tau_z + 1) * e                 = 2z(h-n)        [STT]
        S'  = 2*n + u                         = 2h'            [STT]
  - MLP between GRUs: first linear explicit (leaky relu), second linear
    folded into the next GRU's input matmul (Wih @ W2).
  - lay8 (final MLP+Linear) only needed at t=T-1, computed once after scan.
"""

import numpy as np

H = 40
D = 20
T = 512
B = 2048
NCORES = 8
BS = B // NCORES  # 256
NL = 5            # GRU layers
UNROLL = 16

_F32 = np.float32
LAST_EXEC_NS = None


def _prep_weights(g0_wih, g0_whh, g0_bih, g0_bhh, g_wih, g_whh, g_bih, g_bhh,
                  mlp_w1, mlp_b1, mlp_w2, mlp_b2, w_last, b_last):
    """Host-side fusion/scaling. Returns dict of packed fp32 arrays."""
    wxrz = []    # x-side rz lhsT per layer: [Kx, 104] (r cols 0:40, z 64:104)
    wxn = []     # x-side n  lhsT per layer: [Kx, 40]
    whrz = []    # h-side rz lhsT: [40, 104]
    whn = []     # h-side n lhsT: [41, 40]
    wm = []      # MLP first-linear lhsT: [41, 40]  (3 mlps + head)

    def gru_parts(wih, whh, bih, bhh, fused_w2=None, fused_b2=None,
                  in_is_S=False):
        Weff = wih if fused_w2 is None else wih @ fused_w2     # [120, in_dim]
        beff = (bih if fused_b2 is None else bih + wih @ fused_b2).copy()
        s_in = 0.5 if in_is_S else 1.0   # input arrives as S=2h
        K = Weff.shape[1]
        rz = np.zeros((K + 1, 104), dtype=_F32)
        rz[0:K, 0:40] = 0.5 * s_in * Weff[0:40].T
        rz[0:K, 64:104] = 0.5 * s_in * Weff[40:80].T
        rz[K, 0:40] = 0.5 * (beff[0:40] + bhh[0:40])
        rz[K, 64:104] = 0.5 * (beff[40:80] + bhh[40:80])
        xn = np.zeros((K + 1, 40), dtype=_F32)
        xn[0:K] = s_in * Weff[80:120].T
        xn[K] = beff[80:120]
        hrz = np.zeros((40, 104), dtype=_F32)
        hrz[:, 0:40] = 0.25 * whh[0:40].T
        hrz[:, 64:104] = 0.25 * whh[40:80].T
        hn = np.zeros((41, 40), dtype=_F32)
        hn[0:40] = 0.25 * whh[80:120].T
        hn[40] = 0.5 * bhh[80:120]
        return rz, xn, hrz, hn

    qs = [gru_parts(g0_wih, g0_whh, g0_bih, g0_bhh),
          gru_parts(g_wih[0], g_whh[0], g_bih[0], g_bhh[0], in_is_S=True)]
    for m in range(3):
        qs.append(gru_parts(g_wih[m + 1], g_whh[m + 1], g_bih[m + 1],
                            g_bhh[m + 1],
                            fused_w2=mlp_w2[m], fused_b2=mlp_b2[m]))
    for q in qs:
        wxrz.append(q[0]); wxn.append(q[1]); whrz.append(q[2]); whn.append(q[3])
    for m in range(4):
        wml = np.zeros((41, 40), dtype=_F32)
        wml[0:40] = 0.5 * mlp_w1[m].T
        wml[40] = mlp_b1[m]
        wm.append(wml)

    return dict(
        wxrz=wxrz, wxn=wxn, whrz=whrz, whn=whn, wm=wm,
        wlast=np.ascontiguousarray(w_last.T).astype(_F32),   # [40, 1]
        blast=float(b_last[0]),
    )


def model_numpy(x, W):
    """Pure-numpy emulation of the device math (for validation)."""
    Bn = x.shape[0]
    out = np.zeros((Bn, 1), dtype=_F32)
    xT = np.concatenate([x.transpose(2, 1, 0),
                         np.ones((1, T, Bn), _F32)], axis=0)  # [21, T, B]
    S = [np.zeros((41, Bn), _F32) for _ in range(NL)]
    for s in S:
        s[40] = 1.0
    for t in range(T):
        inp = xT[:, t, :]
        for l in range(NL):
            if l >= 2:
                pa = W['wm'][l - 2].T @ S[l - 1]
                a = np.where(pa >= 0, pa, 0.01 * pa)
                inp = np.concatenate([a, np.ones((1, Bn), _F32)], axis=0)
            elif l == 1:
                inp = S[0]
            grz = W['wxrz'][l].T @ inp                  # [104, B]
            grz += W['whrz'][l].T @ S[l][0:40]
            xn = W['wxn'][l].T @ inp                    # [40, B]
            p = W['whn'][l].T @ S[l]                    # [40, B]
            tau = np.tanh(grz)
            v = (tau[0:40] + 1.0) * p
            t2 = v + xn
            n = np.tanh(t2)
            e = 0.5 * S[l][0:40] - n
            u = (tau[64:104] + 1.0) * e
            S[l][0:40] = 2.0 * n + u
    pa = W['wm'][3].T @ S[4]
    a = np.where(pa >= 0, pa, 0.01 * pa)
    z = W['wlast'].T @ a + W['blast']                   # [1, B]
    out[:, 0] = (1.0 / (1.0 + np.exp(-z)))[0]
    return out


def _build_program(W):
    import concourse.bass as bass
    import concourse.tile as tile
    from concourse import bacc, mybir

    f32 = mybir.dt.float32
    bf16 = mybir.dt.bfloat16
    AF = mybir.ActivationFunctionType
    OP = mybir.AluOpType

    nc = bacc.Bacc("TRN2", target_bir_lowering=False)

    x_dram = nc.dram_tensor("x", [21, T, BS], bf16, kind="ExternalInput")
    wxrz_dram = [nc.dram_tensor(f"wxrz{l}", list(W['wxrz'][l].shape), bf16,
                                kind="ExternalInput") for l in range(NL)]
    wxn_dram = [nc.dram_tensor(f"wxn{l}", list(W['wxn'][l].shape), bf16,
                               kind="ExternalInput") for l in range(NL)]
    whrz_dram = [nc.dram_tensor(f"whrz{l}", [40, 104], bf16,
                                kind="ExternalInput") for l in range(NL)]
    whn_dram = [nc.dram_tensor(f"whn{l}", [41, 40], bf16,
                               kind="ExternalInput") for l in range(NL)]
    wm_dram = [nc.dram_tensor(f"wm{m}", [41, 40], bf16,
                              kind="ExternalInput") for m in range(4)]
    wlast_dram = nc.dram_tensor("wlast", [40, 1], bf16, kind="ExternalInput")
    ones_dram = nc.dram_tensor("ones1", [1, BS], bf16, kind="ExternalInput")
    out_dram = nc.dram_tensor("out", [1, BS], f32, kind="ExternalOutput")

    with tile.TileContext(nc) as tc:
        with (
            tc.tile_pool(name="wpool", bufs=1) as wpool,
            tc.tile_pool(name="spool", bufs=1) as spool,
            tc.tile_pool(name="xpool", bufs=1) as xpool,
            tc.tile_pool(name="work", bufs=2) as work,
            tc.tile_pool(name="psum", bufs=1, space="PSUM") as psum,
        ):
            # --- persistent weight tiles ---
            wxrz_t = [wpool.tile(list(W['wxrz'][l].shape), bf16, tag=f"wxrz{l}",
                                 name=f"wxrz{l}t") for l in range(NL)]
            wxn_t = [wpool.tile(list(W['wxn'][l].shape), bf16, tag=f"wxn{l}",
                                name=f"wxn{l}t") for l in range(NL)]
            whrz_t = [wpool.tile([40, 104], bf16, tag=f"whrz{l}",
                                 name=f"whrz{l}t") for l in range(NL)]
            whn_t = [wpool.tile([41, 40], bf16, tag=f"whn{l}",
                                name=f"whn{l}t") for l in range(NL)]
            wm_t = [wpool.tile([41, 40], bf16, tag=f"wm{m}",
                               name=f"wm{m}t") for m in range(4)]
            wlast_t = wpool.tile([40, 1], bf16, tag="wlast", name="wlastt")
            for l in range(NL):
                nc.sync.dma_start(wxrz_t[l][:], wxrz_dram[l][:])
                nc.sync.dma_start(wxn_t[l][:], wxn_dram[l][:])
                nc.sync.dma_start(whrz_t[l][:], whrz_dram[l][:])
                nc.sync.dma_start(whn_t[l][:], whn_dram[l][:])
            for m in range(4):
                nc.sync.dma_start(wm_t[m][:], wm_dram[m][:])
            nc.sync.dma_start(wlast_t[:], wlast_dram[:])

            # --- state tiles: S = 2h rows 0:40, row 40 = 1.0 (via DMA) ---
            S_t = [spool.tile([41, BS], bf16, tag=f"s{l}", name=f"s{l}")
                   for l in range(NL)]
            for l in range(NL):
                nc.vector.memset(S_t[l][0:40, :], 0.0)
                nc.sync.dma_start(S_t[l][40:41, :], ones_dram[:])

            # --- psum: one bank per GRU layer ---
            pg = [psum.tile([128, 512], f32, tag=f"pg{l}", name=f"pg{l}")
                  for l in range(NL)]
            pm = psum.tile([128, 512], f32, tag="pm", name="pm")

            x_tile = xpool.tile([21, UNROLL * BS], bf16, tag="xc", name="xc")

            def gru_step(l, in_rhs):
                """in_rhs: [Kx, BS] input including ones row."""
                rzA = pg[l][0:104, 0:256]
                xnB = pg[l][0:40, 256:512]
                pB = pg[l][64:104, 256:512]
                nc.tensor.matmul(rzA, wxrz_t[l][:], in_rhs,
                                 start=True, stop=False, skip_group_check=True)
                nc.tensor.matmul(rzA, whrz_t[l][:], S_t[l][0:40, :],
                                 start=False, stop=True, skip_group_check=True)
                nc.tensor.matmul(xnB, wxn_t[l][:], in_rhs,
                                 start=True, stop=True, skip_group_check=True)
                nc.tensor.matmul(pB, whn_t[l][:], S_t[l][:],
                                 start=True, stop=True, skip_group_check=True)
                tau = work.tile([104, BS], bf16, tag=f"tau{l}", name=f"tau{l}")
                nc.scalar.activation(tau[:], rzA, AF.Tanh)
                v = work.tile([40, BS], bf16, tag=f"v{l}", name=f"v{l}")
                nc.vector.scalar_tensor_tensor(
                    v[:], tau[0:40, :], 1.0, pB, OP.add, OP.mult)
                t2 = work.tile([40, BS], bf16, tag=f"t2{l}", name=f"t2{l}")
                nc.vector.tensor_add(t2[:], v[:], xnB)
                n_t = work.tile([40, BS], bf16, tag=f"n{l}", name=f"n{l}")
                nc.scalar.activation(n_t[:], t2[:], AF.Tanh)
                e = work.tile([104, BS], bf16, tag=f"e{l}", name=f"e{l}")
                nc.vector.scalar_tensor_tensor(
                    e[64:104, :], S_t[l][0:40, :], 0.5, n_t[:],
                    OP.mult, OP.subtract)
                u = work.tile([40, BS], bf16, tag=f"u{l}", name=f"u{l}")
                nc.vector.scalar_tensor_tensor(
                    u[:], tau[64:104, :], 1.0, e[64:104, :], OP.add, OP.mult)
                nc.vector.scalar_tensor_tensor(
                    S_t[l][0:40, :], n_t[:], 2.0, u[:], OP.mult, OP.add)

            # MLP psum regions within the shared pm bank
            pm_reg = [pm[0:40, 0:256], pm[64:104, 0:256], pm[0:40, 256:512]]

            def mlp_step(m, a_dst):
                nc.tensor.matmul(pm_reg[m], wm_t[m][:], S_t[m + 1][:],
                                 start=True, stop=True)
                nc.scalar.activation(a_dst[0:40, :], pm_reg[m], AF.Lrelu,
                                     alpha=0.01)

            # a-input tiles for layers C,D,E; ones row via DMA, done once
            a_t = [[work.tile([41, BS], bf16, tag=f"a{m}_{k}",
                              name=f"a{m}_{k}") for k in range(2)]
                   for m in range(3)]
            for m in range(3):
                for k in range(2):
                    nc.sync.dma_start(a_t[m][k][40:41, :], ones_dram[:])

            n_iter = T // UNROLL
            with tc.For_i(0, n_iter, 1,
                          hint_engines=(mybir.EngineType.PE, mybir.EngineType.DVE)) as it:
                nc.sync.dma_start(
                    x_tile[:], x_dram[:, bass.ts(it, UNROLL), :])
                for uu in range(UNROLL):
                    xin = x_tile[:, uu * BS:(uu + 1) * BS]
                    gru_step(0, xin)
                    gru_step(1, S_t[0][:])
                    for m in range(3):
                        ad = a_t[m][uu % 2]
                        mlp_step(m, ad)
                        gru_step(m + 2, ad[:])

            # --- head: lay8 on final S_E ---
            pah = pm[64:104, 256:512]
            nc.tensor.matmul(pah, wm_t[3][:], S_t[4][:], start=True, stop=True)
            a_h = work.tile([40, BS], bf16, tag="ah", name="ah")
            nc.scalar.activation(a_h[:], pah, AF.Lrelu, alpha=0.01)
            pth = pm[32:33, 0:256]
            nc.tensor.matmul(pth, wlast_t[:], a_h[:], start=True, stop=True)
            o_sb = work.tile([1, BS], f32, tag="osb", name="osb")
            blast_t = wpool.tile([1, 1], f32, tag="blast", name="blastt")
            nc.vector.memset(blast_t[:], W['blast'])
            nc.scalar.activation(o_sb[:], pth, AF.Sigmoid, bias=blast_t[:])
            nc.sync.dma_start(out_dram[:], o_sb[:])

    nc.compile()
    return nc, out_dram.name


def _make_in_maps(W, x):
    import ml_dtypes
    bf = ml_dtypes.bfloat16
    wmaps = {}
    for l in range(NL):
        wmaps[f"wxrz{l}"] = W['wxrz'][l].astype(bf)
        wmaps[f"wxn{l}"] = W['wxn'][l].astype(bf)
        wmaps[f"whrz{l}"] = W['whrz'][l].astype(bf)
        wmaps[f"whn{l}"] = W['whn'][l].astype(bf)
    for m in range(4):
        wmaps[f"wm{m}"] = W['wm'][m].astype(bf)
    wmaps["wlast"] = W['wlast'].astype(bf)
    wmaps["ones1"] = np.ones((1, BS), dtype=bf)

    in_maps = []
    for c in range(NCORES):
        xs = x[c * BS:(c + 1) * BS]                    # [BS, T, D]
        xT = np.empty((21, T, BS), dtype=bf)
        xT[0:20] = xs.transpose(2, 1, 0).astype(bf)
        xT[20] = 1.0
        m = dict(wmaps)
        m["x"] = np.ascontiguousarray(xT)
        in_maps.append(m)
    return in_maps


def _gather(out_concat):
    """out_concat: [NCORES*1, BS] concatenated core outputs -> [B, 1]."""
    out = np.empty((B, 1), dtype=_F32)
    for c in range(NCORES):
        out[c * BS:(c + 1) * BS, 0] = out_concat[c]
    return out


def kernel(**inputs):
    import sys
    if '/opt/trn_rl_repo' not in sys.path:
        sys.path.insert(0, '/opt/trn_rl_repo')
    from concourse.bass_utils import run_bass_kernel_spmd

    W = _prep_weights(**{k: np.asarray(v) for k, v in inputs.items()
                         if k != 'x'})
    x = np.asarray(inputs['x'], dtype=_F32)

    nc, out_name = _build_program(W)
    in_maps = _make_in_maps(W, x)

    res = run_bass_kernel_spmd(nc, in_maps, core_ids=list(range(NCORES)))
    global LAST_EXEC_NS
    LAST_EXEC_NS = res.exec_time_ns
    out = np.empty((B, 1), dtype=_F32)
    for c in range(NCORES):
        out[c * BS:(c + 1) * BS, 0] = res.results[c][out_name][0]
    return out
